# revision 17
# baseline (speedup 1.0000x reference)
"""GCN (3x GCNConv + global_mean_pool + linear) on 8 Trainium2 NeuronCores.

Self-contained: hardcoded problem shapes (N=50000, E=800000, H=128, F_IN=11,
G=2048).

Math (per conv layer, PyG GCNConv):
    z[d] = dinv[d] * ( sum_{e:dst=d} dinv[src_e]*x[src_e]  +  dinv[d]*x[d] )
    x' = relu(z @ W + b)          (no relu on layer 3)
with dinv = 1/sqrt(1+indeg). We pre-scale the feature table by dinv (x~ =
dinv*x), so edge contributions need only the dst-side dinv, applied once per
128-node block after PSUM accumulation.

Distribution: nodes (padded to 50176 = 8*49*128) sharded contiguously across
8 cores; each core aggregates its own dst blocks, gathering source rows from
a replicated feature table (AllGather per layer). Pooling partials are
scatter-written to graph rows and AllReduced.

Device pipeline per layer/core:
  dma_gather (grouped, 2 int16-safe table windows) -> one-hot M via
  is_equal(iota_row, dst_rel) on DVE -> PE G^T@M accumulate per dst block in
  PSUM -> dinv*(P + x~T) -> @W -> relu+bias (ACT) -> PE transpose -> DMA to
  shard -> AllGather.
"""
import sys

sys.path.insert(0, "/opt/trn_rl_repo")

import numpy as np

N_NODES = 50000
N_EDGES = 800000
HIDDEN = 128
F_IN = 11
F1 = 64                    # layer-1 table cols padded (44B -> 256B rows)
NUM_CLASSES = 19
NUM_GRAPHS = 2048
NCORES = 8
BLK = 128
NBLK = 49                  # blocks per core
SHARD = NBLK * BLK         # 6272 nodes per core
NPAD = NCORES * SHARD      # 50176
GRP = 4                    # blocks per gather group
LO_END = 17408             # A-window: table[0:32768), idx=src
HI_BASE = NPAD - 32768     # 17408; B-window: table[17408:50176), idx=src-HI_BASE
ACC_S = 512                # core-relative pooling slots (4 tiles of 128)
PD_ROWS = 2176             # padded graph rows for scatter (>=2048, *19 %128==0)

_cache = {}


# --------------------------------------------------------------------------
# host preprocessing
# --------------------------------------------------------------------------
def _preprocess(x, edge_index, batch, W1, b1, W2, b2, W3, b3, Wl, bl):
    src = np.asarray(edge_index[0], dtype=np.int64)
    dst = np.asarray(edge_index[1], dtype=np.int64)
    batch = np.asarray(batch, dtype=np.int64)
    x = np.asarray(x, np.float32)

    x_pad = np.zeros((NPAD, F_IN), np.float32)
    x_pad[:N_NODES] = x
    batch_pad = np.full(NPAD, -1, np.int64)
    batch_pad[:N_NODES] = batch

    # --- in-degree-balanced node permutation within 12-block windows -------
    # (keeps pooling graph-windows narrow while equalizing per-block edge
    #  counts so the uniform SPMD tile budgets waste fewer gather slots)
    indeg = np.bincount(dst, minlength=NPAD).astype(np.int64)
    indeg_lo = np.bincount(dst[src < LO_END], minlength=NPAD).astype(np.int64)
    perm = np.arange(NPAD)
    import os
    W = 12
    for c in range(NCORES if os.environ.get("GCN_BAL", "1") == "1" else 0):
        for w0 in range(0, NBLK, W):
            nb = min(W, NBLK - w0)
            p0 = c * SHARD + w0 * BLK
            ids = perm[p0:p0 + nb * BLK].copy()
            tot, lo = indeg[ids], indeg_lo[ids]
            at = max(tot.sum() / nb, 1.0)
            al = max(lo.sum() / nb, 1.0)
            order = np.argsort(-tot, kind="stable")
            bt = np.zeros(nb)
            blo = np.zeros(nb)
            bcnt = np.zeros(nb, np.int64)
            assign = np.empty(nb * BLK, np.int64)
            for i in order:
                scr = np.maximum((bt + tot[i]) / at, (blo + lo[i]) / al)
                scr[bcnt >= BLK] = np.inf
                b = int(np.argmin(scr))
                assign[i] = b
                bt[b] += tot[i]
                blo[b] += lo[i]
                bcnt[b] += 1
            perm[p0:p0 + nb * BLK] = np.concatenate(
                [ids[assign == b] for b in range(nb)])
    inv = np.empty(NPAD, np.int64)
    inv[perm] = np.arange(NPAD)
    src = inv[src]
    dst = inv[dst]
    x_pad = x_pad[perm]
    batch_pad = batch_pad[perm]

    deg = 1.0 + np.bincount(dst, minlength=NPAD).astype(np.float32)
    dinv_pad = (1.0 / np.sqrt(deg)).astype(np.float32)
    dinv_loc = dinv_pad.reshape(NCORES, 1, SHARD)

    table1 = np.zeros((NPAD, F1), np.float32)
    table1[:, :F_IN] = x_pad * dinv_pad[:, None]

    xT_l1 = np.zeros((NCORES, F1, SHARD), np.float32)
    for c in range(NCORES):
        xT_l1[c] = table1[c * SHARD:(c + 1) * SHARD, :].T

    # --- edge grouping -----------------------------------------------------
    core_of = dst // SHARD
    blk_of = (dst % SHARD) // BLK
    rel_of = (dst % BLK).astype(np.float32)
    gblk = core_of * NBLK + blk_of
    cls = np.where(src < LO_END, 0, np.where(src >= 32768, 2, 1)).astype(np.int8)

    nblk_g = NCORES * NBLK
    n_lo = np.bincount(gblk[cls == 0], minlength=nblk_g)
    n_mid = np.bincount(gblk[cls == 1], minlength=nblk_g)
    n_hi = np.bincount(gblk[cls == 2], minlength=nblk_g)

    T_A = max(1, int(np.max(-(-n_lo // BLK))))
    a_fill = np.minimum(n_mid, T_A * BLK - n_lo)
    T_B = max(1, int(np.max(-(-(n_hi + n_mid - a_fill) // BLK))))
    ntile = T_A + T_B
    slots_core = NBLK * ntile * BLK

    order = np.lexsort((cls, gblk))
    src_o, rel_o, cls_o = src[order], rel_of[order], cls[order]
    blk_starts = np.searchsorted(gblk[order], np.arange(nblk_g + 1))

    idx_all = np.zeros((NCORES, slots_core), np.int16)
    dstrel_all = np.full((NCORES, slots_core), 255.0, np.float32)
    for c in range(NCORES):
        for run in (0, 1):
            T_r = T_A if run == 0 else T_B
            base0 = 0 if run == 0 else NBLK * T_A * BLK
            for b in range(NBLK):
                g = c * NBLK + b
                s, e = blk_starts[g], blk_starts[g + 1]
                bsrc, brel, bcls = src_o[s:e], rel_o[s:e], cls_o[s:e]
                a = int(a_fill[g])
                mid_idx = np.nonzero(bcls == 1)[0]
                if run == 0:
                    sel = np.concatenate([np.nonzero(bcls == 0)[0], mid_idx[:a]])
                    iv = bsrc[sel]
                else:
                    sel = np.concatenate([mid_idx[a:], np.nonzero(bcls == 2)[0]])
                    iv = bsrc[sel] - HI_BASE
                k = len(sel)
                assert k <= T_r * BLK
                pos = base0 + b * T_r * BLK
                idx_all[c, pos:pos + k] = iv.astype(np.int16)
                dstrel_all[c, pos:pos + k] = brel[sel]

    idx16 = np.zeros((NCORES, 128, slots_core // 16), np.int16)
    dstrel = np.zeros((NCORES, 128, slots_core // BLK), np.float32)
    for c in range(NCORES):
        idx16[c] = np.tile(idx_all[c].reshape(-1, 16).T, (8, 1))
        dstrel[c] = dstrel_all[c].reshape(-1, BLK).T

    # --- pooling -----------------------------------------------------------
    cnt = np.bincount(batch, minlength=NUM_GRAPHS).astype(np.float32)
    inv_cnt = (1.0 / np.maximum(cnt, 1.0)).astype(np.float32)
    bp = batch_pad.reshape(NCORES, SHARD)
    gc_lo = np.array([int(bp[c][bp[c] >= 0].min()) for c in range(NCORES)])

    # uniform (SPMD) core-relative window base per block: cover all cores
    lo_need = np.full(NBLK, 10 ** 9, np.int64)
    hi_need = np.full(NBLK, 0, np.int64)
    for c in range(NCORES):
        for b in range(NBLK):
            nodes = bp[c, b * BLK:(b + 1) * BLK]
            real = nodes[nodes >= 0]
            if len(real):
                lo_need[b] = min(lo_need[b], real.min() - gc_lo[c])
                hi_need[b] = max(hi_need[b], real.max() - gc_lo[c])
    u_of = np.clip(lo_need, 0, ACC_S - BLK)
    assert (hi_need - u_of).max() < BLK and hi_need.max() < ACC_S

    Bmat = np.zeros((NCORES, 128, NBLK * BLK), np.float32)
    for c in range(NCORES):
        for b in range(NBLK):
            nodes = bp[c, b * BLK:(b + 1) * BLK]
            p = np.nonzero(nodes >= 0)[0]
            if len(p) == 0:
                continue
            s = nodes[p] - gc_lo[c] - u_of[b]
            assert (s >= 0).all() and (s < BLK).all(), (c, b, s.min(), s.max())
            Bmat[c, p, b * BLK + s] = inv_cnt[nodes[p]]

    # absolute graph row per core-relative slot; dummies -> pad rows
    gidx = np.zeros((NCORES, 128, 4), np.int32)
    covered = np.zeros((NCORES, ACC_S), bool)
    for c in range(NCORES):
        for k in range(4):
            g_abs = gc_lo[c] + k * 128 + np.arange(128)
            ok = g_abs < NUM_GRAPHS
            gidx[c, :, k] = np.where(ok, g_abs, 2100)
            covered[c, k * 128:(k + 1) * 128] = ok

    # bias: designate exactly one (core, slot) per graph
    biasmat = np.zeros((NCORES, NUM_CLASSES, ACC_S), np.float32)
    bl32 = np.asarray(bl, np.float32)
    done = np.zeros(NUM_GRAPHS, bool)
    for c in range(NCORES):
        for sl in range(ACC_S):
            if covered[c, sl]:
                g = gc_lo[c] + sl
                if not done[g]:
                    done[g] = True
                    biasmat[c, :, sl] = bl32
    assert done.all()

    wts = dict(
        W1p=np.zeros((F1, HIDDEN), np.float32),
        W2=np.asarray(W2, np.float32), W3=np.asarray(W3, np.float32),
        Wl=np.asarray(Wl, np.float32),
        b1=np.asarray(b1, np.float32).reshape(HIDDEN, 1),
        b2=np.asarray(b2, np.float32).reshape(HIDDEN, 1),
        b3=np.asarray(b3, np.float32).reshape(HIDDEN, 1),
    )
    wts["W1p"][:F_IN] = np.asarray(W1, np.float32)

    meta = dict(T_A=T_A, T_B=T_B, slots_core=slots_core, u_of=u_of)
    per_core = [dict(idx16=idx16[c], dstrel=dstrel[c], xT_l1=xT_l1[c],
                     dinv_loc=dinv_loc[c], Bmat=Bmat[c], gidx=gidx[c],
                     biasmat=biasmat[c], table1=table1, **wts)
                for c in range(NCORES)]
    return meta, per_core


# --------------------------------------------------------------------------
# device program
# --------------------------------------------------------------------------
def _build(meta, repeat=1):
    import concourse.bacc as bacc
    import concourse.bass as bass
    import concourse.tile as tile
    from concourse import mybir
    from concourse.masks import make_identity

    T_A, T_B = meta["T_A"], meta["T_B"]
    slots = meta["slots_core"]
    f32 = mybir.dt.float32

    import os
    scr = int(os.environ.get("GCN_SCR", "16384"))
    nc = bacc.Bacc("TRN2", target_bir_lowering=False, debug=False,
                   num_devices=NCORES, dynamic_dma_scratch_size=scr)
    ti = lambda n, s, d=f32: nc.dram_tensor(n, s, d, kind="ExternalInput")
    table1 = ti("table1", [NPAD, F1])
    idx16 = ti("idx16", [128, slots // 16], mybir.dt.int16)
    dstrel = ti("dstrel", [128, slots // BLK])
    xT_l1 = ti("xT_l1", [F1, SHARD])
    dinv_loc = ti("dinv_loc", [1, SHARD])
    Bmat_d = ti("Bmat", [128, NBLK * BLK])
    gidx_d = ti("gidx", [128, 4], mybir.dt.int32)
    biasmat_d = ti("biasmat", [NUM_CLASSES, ACC_S])
    W1p_d, W2_d, W3_d = ti("W1p", [F1, HIDDEN]), ti("W2", [HIDDEN, HIDDEN]), ti("W3", [HIDDEN, HIDDEN])
    Wl_d = ti("Wl", [HIDDEN, NUM_CLASSES])
    b1_d, b2_d, b3_d = ti("b1", [HIDDEN, 1]), ti("b2", [HIDDEN, 1]), ti("b3", [HIDDEN, 1])
    out_d = nc.dram_tensor("out", [NUM_GRAPHS, NUM_CLASSES], f32,
                           kind="ExternalOutput")

    with tile.TileContext(nc) as tc:
        with (
            tc.tile_pool(name="const", bufs=1) as cp,
            tc.tile_pool(name="work", bufs=1) as wp,
            tc.tile_pool(name="ps", bufs=2, space="PSUM") as ps,
            tc.tile_pool(name="dram", bufs=1, space="DRAM") as dp,
        ):
            # ---- constants / persistent state ----
            idx_sb = cp.tile([128, slots // 16], mybir.dt.int16)
            nc.sync.dma_start(idx_sb[:], idx16[:])
            dst_sb = cp.tile([128, slots // BLK], f32)
            nc.sync.dma_start(dst_sb[:], dstrel[:])
            dinv_rep = cp.tile([128, SHARD], f32)
            nc.sync.dma_start(dinv_rep[:], dinv_loc[:].to_broadcast([128, SHARD]))
            xT_cur = cp.tile([128, SHARD], f32)
            nc.sync.dma_start(xT_cur[:F1, :], xT_l1[:])
            iota_row = cp.tile([128, 128], f32)
            nc.gpsimd.iota(iota_row[:], pattern=[[1, 128]], base=0,
                           channel_multiplier=0,
                           allow_small_or_imprecise_dtypes=True)
            ident = cp.tile([128, 128], f32)
            make_identity(nc, ident[:])
            W1p = cp.tile([F1, HIDDEN], f32)
            nc.sync.dma_start(W1p[:], W1p_d[:])
            W2 = cp.tile([HIDDEN, HIDDEN], f32)
            nc.sync.dma_start(W2[:], W2_d[:])
            W3 = cp.tile([HIDDEN, HIDDEN], f32)
            nc.sync.dma_start(W3[:], W3_d[:])
            Wl = cp.tile([HIDDEN, NUM_CLASSES], f32)
            nc.sync.dma_start(Wl[:], Wl_d[:])
            b1 = cp.tile([HIDDEN, 1], f32)
            nc.sync.dma_start(b1[:], b1_d[:])
            b2 = cp.tile([HIDDEN, 1], f32)
            nc.sync.dma_start(b2[:], b2_d[:])
            b3 = cp.tile([HIDDEN, 1], f32)
            nc.sync.dma_start(b3[:], b3_d[:])
            accT = cp.tile([128, ACC_S], f32)
            nc.vector.memset(accT[:], 0.0)


            u_of = meta["u_of"]

            import os
            # gather chunk in tiles of 128 descriptors; <=896 descriptors per
            # dma_gather (SWDGE descriptor-ring capacity is ~1024)
            CH = int(os.environ.get("GCN_CH", "4"))

            def layer(lnum, tbl, F, W_sb, b_sb, ag_in, ag_out):
                role = (lnum - 1) % 3 + 1
                nA, nB = NBLK * T_A, NBLK * T_B
                aCH = [(s, min(s + CH, nA)) for s in range(0, nA, CH)]
                bCH = [(s, min(s + CH, nB)) for s in range(0, nB, CH)]
                ga, gb = {}, {}
                ai = bi = 0
                for b in range(NBLK):
                    while ai < len(aCH) and aCH[ai][0] < (b + 1) * T_A:
                        s, e = aCH[ai]
                        gt = wp.tile([128, e - s, F], f32, tag="gA", bufs=3,
                                     name=f"gA_{lnum}_{ai}")
                        nc.gpsimd.dma_gather(
                            gt[:], tbl[0:32768, :], idx_sb[:, s * 8:e * 8],
                            (e - s) * BLK, (e - s) * BLK, F)
                        ga[ai] = gt
                        ai += 1
                    while bi < len(bCH) and bCH[bi][0] < (b + 1) * T_B:
                        s, e = bCH[bi]
                        gt = wp.tile([128, e - s, F], f32, tag="gB", bufs=4,
                                     name=f"gB_{lnum}_{bi}")
                        nc.gpsimd.dma_gather(
                            gt[:], tbl[HI_BASE:NPAD, :],
                            idx_sb[:, nA * 8 + s * 8:nA * 8 + e * 8],
                            (e - s) * BLK, (e - s) * BLK, F)
                        gb[bi] = gt
                        bi += 1
                    pz = ps.tile([F, 128], f32, tag="pz", bufs=2,
                                 name=f"pz_{lnum}_{b}")
                    nt = 0
                    for run, gmap, T_r, col0 in (
                        (0, ga, T_A, b * T_A),
                        (1, gb, T_B, b * T_B),
                    ):
                        for t in range(T_r):
                            j = col0 + t                  # stream tile index
                            chunk, sl = j // CH, j % CH
                            dcol = j if run == 0 else nA + j
                            m = wp.tile([128, 128], f32, tag="m", bufs=4,
                                        name=f"m_{lnum}_{b}_{run}_{t}")
                            nc.vector.tensor_scalar(
                                out=m[:], in0=iota_row[:],
                                scalar1=dst_sb[:, dcol:dcol + 1],
                                scalar2=None, op0=mybir.AluOpType.is_equal)
                            nc.tensor.matmul(
                                pz[:], lhsT=gmap[chunk][:, sl, :], rhs=m[:],
                                start=(nt == 0), stop=(nt == ntile_tot - 1))
                            nt += 1
                    if True:
                        blk_sl = slice(b * BLK, (b + 1) * BLK)
                        s_sb = wp.tile([F, 128], f32, tag="s", bufs=2,
                                       name=f"s_{lnum}_{b}")
                        nc.vector.tensor_tensor(out=s_sb[:], in0=pz[:],
                                                in1=xT_cur[:F, blk_sl],
                                                op=mybir.AluOpType.add)
                        zt = wp.tile([F, 128], f32, tag="zt", bufs=2,
                                     name=f"zt_{lnum}_{b}")
                        nc.vector.tensor_tensor(out=zt[:], in0=s_sb[:],
                                                in1=dinv_rep[:F, blk_sl],
                                                op=mybir.AluOpType.mult)
                        pxn = ps.tile([HIDDEN, 128], f32, tag="pxn", bufs=2,
                                      name=f"pxn_{lnum}_{b}")
                        nc.tensor.matmul(pxn[:], lhsT=W_sb[:], rhs=zt[:],
                                         start=True, stop=True)
                        if role < 3:
                            xnT = wp.tile([HIDDEN, 128], f32, tag="xnT", bufs=2,
                                          name=f"xnT_{lnum}_{b}")
                            nc.scalar.activation(xnT[:], pxn[:],
                                                 mybir.ActivationFunctionType.Relu,
                                                 bias=b_sb[:])
                            nc.vector.tensor_tensor(out=xT_cur[:, blk_sl],
                                                    in0=xnT[:],
                                                    in1=dinv_rep[:, blk_sl],
                                                    op=mybir.AluOpType.mult)
                            ptr = ps.tile([128, HIDDEN], f32, tag="ptr", bufs=2,
                                          name=f"ptr_{lnum}_{b}")
                            nc.tensor.transpose(ptr[:], xT_cur[:, blk_sl], ident[:])
                            tr = wp.tile([128, HIDDEN], f32, tag="tr", bufs=2,
                                         name=f"tr_{lnum}_{b}")
                            nc.scalar.copy(tr[:], ptr[:])
                            nc.sync.dma_start(ag_in[blk_sl, :], tr[:])
                        else:
                            h3T = wp.tile([HIDDEN, 128], f32, tag="xnT", bufs=2,
                                          name=f"h3T_{b}")
                            nc.vector.tensor_scalar(
                                out=h3T[:], in0=pxn[:], scalar1=b_sb[:],
                                scalar2=None, op0=mybir.AluOpType.add)
                            ptr = ps.tile([128, HIDDEN], f32, tag="ptr", bufs=2,
                                          name=f"ptr3_{b}")
                            nc.tensor.transpose(ptr[:], h3T[:], ident[:])
                            tr = wp.tile([128, HIDDEN], f32, tag="tr", bufs=2,
                                         name=f"tr3_{b}")
                            nc.scalar.copy(tr[:], ptr[:])
                            bt = wp.tile([128, BLK], f32, tag="bt", bufs=4,
                                         name=f"bt_{b}")
                            nc.sync.dma_start(bt[:], Bmat_d[:, b * BLK:(b + 1) * BLK])
                            pp = ps.tile([128, HIDDEN], f32, tag="ptr", bufs=2,
                                         name=f"pp_{b}")
                            nc.tensor.matmul(pp[:], lhsT=tr[:], rhs=bt[:],
                                             start=True, stop=True)
                            u = int(u_of[b])
                            nc.vector.tensor_tensor(
                                out=accT[:, u:u + BLK], in0=accT[:, u:u + BLK],
                                in1=pp[:], op=mybir.AluOpType.add)

                if role < 3:
                    nc.gpsimd.collective_compute(
                        "AllGather", mybir.AluOpType.bypass,
                        replica_groups=[list(range(NCORES))],
                        ins=[ag_in[:]], outs=[ag_out[:]])

            ntile_tot = T_A + T_B
            ag_in1 = dp.tile([SHARD, HIDDEN], f32)
            ag_in2 = dp.tile([SHARD, HIDDEN], f32)
            gidx_sb = cp.tile([128, 4], mybir.dt.int32)
            nc.sync.dma_start(gidx_sb[:], gidx_d[:])
            biasm_sb = cp.tile([NUM_CLASSES, ACC_S], f32)
            nc.sync.dma_start(biasm_sb[:], biasmat_d[:])
            pd = dp.tile([PD_ROWS, NUM_CLASSES], f32)
            zt19 = wp.tile([128, PD_ROWS * NUM_CLASSES // 128], f32)
            nc.vector.memset(zt19[:], 0.0)

            for rep in range(repeat):
              ln1, ln2, ln3 = 3 * rep + 1, 3 * rep + 2, 3 * rep + 3
              ag1_out = dp.tile([NPAD, HIDDEN], f32, addr_space="Shared",
                                name=f"ag1_out_{rep}", tag=f"ag1_{rep}")
              ag2_out = dp.tile([NPAD, HIDDEN], f32, addr_space="Shared",
                                name=f"ag2_out_{rep}", tag=f"ag2_{rep}")
              if rep > 0:
                nc.vector.memset(accT[:], 0.0)
              layer(ln1, table1, F1, W1p, b1, ag_in1, ag1_out)
              layer(ln2, ag1_out, HIDDEN, W2, b2, ag_in2, ag2_out)
              layer(ln3, ag2_out, HIDDEN, W3, b3, None, None)

              # ---- pooling tail ----
              nc.sync.dma_start(
                pd[:].rearrange("a b -> (a b)").rearrange("(p f) -> p f", p=128),
                zt19[:])

              for k in range(4):
                py = ps.tile([NUM_CLASSES, 128], f32, tag="ptr", bufs=2,
                             name=f"py_{rep}_{k}")
                nc.tensor.matmul(py[:], lhsT=Wl[:],
                                 rhs=accT[:, k * 128:(k + 1) * 128],
                                 start=True, stop=True)
                y = wp.tile([NUM_CLASSES, 128], f32, tag="s", bufs=2,
                            name=f"y_{rep}_{k}")
                nc.vector.tensor_tensor(
                    out=y[:], in0=py[:],
                    in1=biasm_sb[:, k * 128:(k + 1) * 128],
                    op=mybir.AluOpType.add)
                pyt = ps.tile([128, NUM_CLASSES], f32, tag="pz", bufs=2,
                              name=f"pyt_{rep}_{k}")
                nc.tensor.transpose(pyt[:], y[:], ident[:NUM_CLASSES, :NUM_CLASSES])
                yT = wp.tile([128, NUM_CLASSES], f32, tag="zt", bufs=2,
                             name=f"yT_{rep}_{k}")
                nc.scalar.copy(yT[:], pyt[:])
                nc.gpsimd.indirect_dma_start(
                    out=pd[:],
                    out_offset=bass.IndirectOffsetOnAxis(ap=gidx_sb[:, k:k + 1],
                                                         axis=0),
                    in_=yT[:], in_offset=None)

              pd_red = dp.tile([PD_ROWS, NUM_CLASSES], f32,
                               addr_space="Shared", name=f"pd_red_{rep}",
                               tag=f"pdr_{rep}")
              nc.gpsimd.collective_compute(
                "AllReduce", mybir.AluOpType.add,
                replica_groups=[list(range(NCORES))],
                ins=[pd[:]], outs=[pd_red[:]])
              nc.sync.dma_start(out_d[:], pd_red[0:NUM_GRAPHS, :])

    nc.compile()
    return nc


# --------------------------------------------------------------------------
def kernel(**inputs):
    from concourse import bass_utils

    meta, per_core = _preprocess(**inputs)
    key = (meta["T_A"], meta["T_B"])
    if key not in _cache:
        _cache[key] = _build(meta)
    nc = _cache[key]
    res = bass_utils.run_bass_kernel_spmd(nc, per_core,
                                          core_ids=list(range(NCORES)))
    return np.asarray(res.results[0]["out"], np.float32)


# revision 18
# speedup vs baseline: 1.0338x; 1.0338x over previous
"""GCN (3x GCNConv + global_mean_pool + linear) on 8 Trainium2 NeuronCores.

Self-contained: hardcoded problem shapes (N=50000, E=800000, H=128, F_IN=11,
G=2048).

Math (per conv layer, PyG GCNConv):
    z[d] = dinv[d] * ( sum_{e:dst=d} dinv[src_e]*x[src_e]  +  dinv[d]*x[d] )
    x' = relu(z @ W + b)          (no relu on layer 3)
with dinv = 1/sqrt(1+indeg). We pre-scale the feature table by dinv (x~ =
dinv*x), so edge contributions need only the dst-side dinv, applied once per
128-node block after PSUM accumulation.

Distribution: nodes (padded to 50176 = 8*49*128) sharded contiguously across
8 cores; each core aggregates its own dst blocks, gathering source rows from
a replicated feature table (AllGather per layer). Pooling partials are
scatter-written to graph rows and AllReduced.

Device pipeline per layer/core:
  dma_gather (grouped, 2 int16-safe table windows) -> one-hot M via
  is_equal(iota_row, dst_rel) on DVE -> PE G^T@M accumulate per dst block in
  PSUM -> dinv*(P + x~T) -> @W -> relu+bias (ACT) -> PE transpose -> DMA to
  shard -> AllGather.
"""
import sys

sys.path.insert(0, "/opt/trn_rl_repo")

import numpy as np

N_NODES = 50000
N_EDGES = 800000
HIDDEN = 128
F_IN = 11
F1 = 64                    # layer-1 table cols padded (44B -> 256B rows)
NUM_CLASSES = 19
NUM_GRAPHS = 2048
NCORES = 8
BLK = 128
NBLK = 49                  # blocks per core
SHARD = NBLK * BLK         # 6272 nodes per core
NPAD = NCORES * SHARD      # 50176
GRP = 4                    # blocks per gather group
LO_END = 17408             # A-window: table[0:32768), idx=src
HI_BASE = NPAD - 32768     # 17408; B-window: table[17408:50176), idx=src-HI_BASE
ACC_S = 512                # core-relative pooling slots (4 tiles of 128)
PD_ROWS = 2176             # padded graph rows for scatter (>=2048, *19 %128==0)

_cache = {}


# --------------------------------------------------------------------------
# host preprocessing
# --------------------------------------------------------------------------
def _preprocess(x, edge_index, batch, W1, b1, W2, b2, W3, b3, Wl, bl):
    src = np.asarray(edge_index[0], dtype=np.int64)
    dst = np.asarray(edge_index[1], dtype=np.int64)
    batch = np.asarray(batch, dtype=np.int64)
    x = np.asarray(x, np.float32)

    x_pad = np.zeros((NPAD, F_IN), np.float32)
    x_pad[:N_NODES] = x
    batch_pad = np.full(NPAD, -1, np.int64)
    batch_pad[:N_NODES] = batch

    # --- in-degree-balanced node permutation within 12-block windows -------
    # (keeps pooling graph-windows narrow while equalizing per-block edge
    #  counts so the uniform SPMD tile budgets waste fewer gather slots)
    indeg = np.bincount(dst, minlength=NPAD).astype(np.int64)
    indeg_lo = np.bincount(dst[src < LO_END], minlength=NPAD).astype(np.int64)
    perm = np.arange(NPAD)
    import os
    W = 12
    for c in range(NCORES if os.environ.get("GCN_BAL", "1") == "1" else 0):
        for w0 in range(0, NBLK, W):
            nb = min(W, NBLK - w0)
            p0 = c * SHARD + w0 * BLK
            ids = perm[p0:p0 + nb * BLK].copy()
            tot, lo = indeg[ids], indeg_lo[ids]
            at = max(tot.sum() / nb, 1.0)
            al = max(lo.sum() / nb, 1.0)
            order = np.argsort(-tot, kind="stable")
            bt = np.zeros(nb)
            blo = np.zeros(nb)
            bcnt = np.zeros(nb, np.int64)
            assign = np.empty(nb * BLK, np.int64)
            for i in order:
                scr = np.maximum((bt + tot[i]) / at, (blo + lo[i]) / al)
                scr[bcnt >= BLK] = np.inf
                b = int(np.argmin(scr))
                assign[i] = b
                bt[b] += tot[i]
                blo[b] += lo[i]
                bcnt[b] += 1
            perm[p0:p0 + nb * BLK] = np.concatenate(
                [ids[assign == b] for b in range(nb)])
    inv = np.empty(NPAD, np.int64)
    inv[perm] = np.arange(NPAD)
    src = inv[src]
    dst = inv[dst]
    x_pad = x_pad[perm]
    batch_pad = batch_pad[perm]

    deg = 1.0 + np.bincount(dst, minlength=NPAD).astype(np.float32)
    dinv_pad = (1.0 / np.sqrt(deg)).astype(np.float32)
    dinv_loc = dinv_pad.reshape(NCORES, 1, SHARD)

    table1 = np.zeros((NPAD, F1), np.float32)
    table1[:, :F_IN] = x_pad * dinv_pad[:, None]

    xT_l1 = np.zeros((NCORES, F1, SHARD), np.float32)
    for c in range(NCORES):
        xT_l1[c] = table1[c * SHARD:(c + 1) * SHARD, :].T

    # --- edge grouping -----------------------------------------------------
    core_of = dst // SHARD
    blk_of = (dst % SHARD) // BLK
    rel_of = (dst % BLK).astype(np.float32)
    gblk = core_of * NBLK + blk_of
    cls = np.where(src < LO_END, 0, np.where(src >= 32768, 2, 1)).astype(np.int8)

    nblk_g = NCORES * NBLK
    n_lo = np.bincount(gblk[cls == 0], minlength=nblk_g)
    n_mid = np.bincount(gblk[cls == 1], minlength=nblk_g)
    n_hi = np.bincount(gblk[cls == 2], minlength=nblk_g)

    T_A = max(1, int(np.max(-(-n_lo // BLK))))
    a_fill = np.minimum(n_mid, T_A * BLK - n_lo)
    T_B = max(1, int(np.max(-(-(n_hi + n_mid - a_fill) // BLK))))
    ntile = T_A + T_B
    slots_core = NBLK * ntile * BLK

    order = np.lexsort((cls, gblk))
    src_o, rel_o, cls_o = src[order], rel_of[order], cls[order]
    blk_starts = np.searchsorted(gblk[order], np.arange(nblk_g + 1))

    idx_all = np.zeros((NCORES, slots_core), np.int16)
    dstrel_all = np.full((NCORES, slots_core), 255.0, np.float32)
    for c in range(NCORES):
        for run in (0, 1):
            T_r = T_A if run == 0 else T_B
            base0 = 0 if run == 0 else NBLK * T_A * BLK
            for b in range(NBLK):
                g = c * NBLK + b
                s, e = blk_starts[g], blk_starts[g + 1]
                bsrc, brel, bcls = src_o[s:e], rel_o[s:e], cls_o[s:e]
                a = int(a_fill[g])
                mid_idx = np.nonzero(bcls == 1)[0]
                if run == 0:
                    sel = np.concatenate([np.nonzero(bcls == 0)[0], mid_idx[:a]])
                    iv = bsrc[sel]
                else:
                    sel = np.concatenate([mid_idx[a:], np.nonzero(bcls == 2)[0]])
                    iv = bsrc[sel] - HI_BASE
                k = len(sel)
                assert k <= T_r * BLK
                pos = base0 + b * T_r * BLK
                idx_all[c, pos:pos + k] = iv.astype(np.int16)
                dstrel_all[c, pos:pos + k] = brel[sel]

    idx16 = np.zeros((NCORES, 128, slots_core // 16), np.int16)
    dstrel = np.zeros((NCORES, 128, slots_core // BLK), np.float32)
    for c in range(NCORES):
        idx16[c] = np.tile(idx_all[c].reshape(-1, 16).T, (8, 1))
        dstrel[c] = dstrel_all[c].reshape(-1, BLK).T

    # --- pooling -----------------------------------------------------------
    cnt = np.bincount(batch, minlength=NUM_GRAPHS).astype(np.float32)
    inv_cnt = (1.0 / np.maximum(cnt, 1.0)).astype(np.float32)
    bp = batch_pad.reshape(NCORES, SHARD)
    gc_lo = np.array([int(bp[c][bp[c] >= 0].min()) for c in range(NCORES)])

    # uniform (SPMD) core-relative window base per block: cover all cores
    lo_need = np.full(NBLK, 10 ** 9, np.int64)
    hi_need = np.full(NBLK, 0, np.int64)
    for c in range(NCORES):
        for b in range(NBLK):
            nodes = bp[c, b * BLK:(b + 1) * BLK]
            real = nodes[nodes >= 0]
            if len(real):
                lo_need[b] = min(lo_need[b], real.min() - gc_lo[c])
                hi_need[b] = max(hi_need[b], real.max() - gc_lo[c])
    u_of = np.clip(lo_need, 0, ACC_S - BLK)
    assert (hi_need - u_of).max() < BLK and hi_need.max() < ACC_S

    Bmat = np.zeros((NCORES, 128, NBLK * BLK), np.float32)
    for c in range(NCORES):
        for b in range(NBLK):
            nodes = bp[c, b * BLK:(b + 1) * BLK]
            p = np.nonzero(nodes >= 0)[0]
            if len(p) == 0:
                continue
            s = nodes[p] - gc_lo[c] - u_of[b]
            assert (s >= 0).all() and (s < BLK).all(), (c, b, s.min(), s.max())
            Bmat[c, p, b * BLK + s] = inv_cnt[nodes[p]]

    # absolute graph row per core-relative slot; dummies -> pad rows
    gidx = np.zeros((NCORES, 128, 4), np.int32)
    covered = np.zeros((NCORES, ACC_S), bool)
    for c in range(NCORES):
        for k in range(4):
            g_abs = gc_lo[c] + k * 128 + np.arange(128)
            ok = g_abs < NUM_GRAPHS
            gidx[c, :, k] = np.where(ok, g_abs, 2100)
            covered[c, k * 128:(k + 1) * 128] = ok

    # bias: designate exactly one (core, slot) per graph
    biasmat = np.zeros((NCORES, NUM_CLASSES, ACC_S), np.float32)
    bl32 = np.asarray(bl, np.float32)
    done = np.zeros(NUM_GRAPHS, bool)
    for c in range(NCORES):
        for sl in range(ACC_S):
            if covered[c, sl]:
                g = gc_lo[c] + sl
                if not done[g]:
                    done[g] = True
                    biasmat[c, :, sl] = bl32
    assert done.all()

    wts = dict(
        W1p=np.zeros((F1, HIDDEN), np.float32),
        W2=np.asarray(W2, np.float32), W3=np.asarray(W3, np.float32),
        Wl=np.asarray(Wl, np.float32),
        b1=np.asarray(b1, np.float32).reshape(HIDDEN, 1),
        b2=np.asarray(b2, np.float32).reshape(HIDDEN, 1),
        b3=np.asarray(b3, np.float32).reshape(HIDDEN, 1),
    )
    wts["W1p"][:F_IN] = np.asarray(W1, np.float32)

    meta = dict(T_A=T_A, T_B=T_B, slots_core=slots_core, u_of=u_of)
    per_core = [dict(idx16=idx16[c], dstrel=dstrel[c], xT_l1=xT_l1[c],
                     dinv_loc=dinv_loc[c], Bmat=Bmat[c], gidx=gidx[c],
                     biasmat=biasmat[c], table1=table1, **wts)
                for c in range(NCORES)]
    return meta, per_core


# --------------------------------------------------------------------------
# device program
# --------------------------------------------------------------------------
def _build(meta, repeat=1):
    import concourse.bacc as bacc
    import concourse.bass as bass
    import concourse.tile as tile
    from concourse import mybir
    from concourse.masks import make_identity

    T_A, T_B = meta["T_A"], meta["T_B"]
    slots = meta["slots_core"]
    f32 = mybir.dt.float32

    import os
    scr = int(os.environ.get("GCN_SCR", "16384"))
    nc = bacc.Bacc("TRN2", target_bir_lowering=False, debug=False,
                   num_devices=NCORES, dynamic_dma_scratch_size=scr)
    ti = lambda n, s, d=f32: nc.dram_tensor(n, s, d, kind="ExternalInput")
    table1 = ti("table1", [NPAD, F1])
    idx16 = ti("idx16", [128, slots // 16], mybir.dt.int16)
    dstrel = ti("dstrel", [128, slots // BLK])
    xT_l1 = ti("xT_l1", [F1, SHARD])
    dinv_loc = ti("dinv_loc", [1, SHARD])
    Bmat_d = ti("Bmat", [128, NBLK * BLK])
    gidx_d = ti("gidx", [128, 4], mybir.dt.int32)
    biasmat_d = ti("biasmat", [NUM_CLASSES, ACC_S])
    W1p_d, W2_d, W3_d = ti("W1p", [F1, HIDDEN]), ti("W2", [HIDDEN, HIDDEN]), ti("W3", [HIDDEN, HIDDEN])
    Wl_d = ti("Wl", [HIDDEN, NUM_CLASSES])
    b1_d, b2_d, b3_d = ti("b1", [HIDDEN, 1]), ti("b2", [HIDDEN, 1]), ti("b3", [HIDDEN, 1])
    out_d = nc.dram_tensor("out", [NUM_GRAPHS, NUM_CLASSES], f32,
                           kind="ExternalOutput")

    with tile.TileContext(nc) as tc:
        with (
            tc.tile_pool(name="const", bufs=1) as cp,
            tc.tile_pool(name="work", bufs=1) as wp,
            tc.tile_pool(name="ps", bufs=2, space="PSUM") as ps,
            tc.tile_pool(name="dram", bufs=1, space="DRAM") as dp,
        ):
            # ---- constants / persistent state ----
            idx_sb = cp.tile([128, slots // 16], mybir.dt.int16)
            nc.sync.dma_start(idx_sb[:], idx16[:])
            M_ON_ACT = os.environ.get("GCN_MACT", "0") == "1"
            dst_sb = cp.tile([128, slots // BLK], f32)
            nc.sync.dma_start(dst_sb[:], dstrel[:])
            negdst_sb = cp.tile([128, slots // BLK], f32)
            nc.vector.tensor_scalar(out=negdst_sb[:], in0=dst_sb[:],
                                    scalar1=-1.0, scalar2=None,
                                    op0=mybir.AluOpType.mult)
            dinv_rep = cp.tile([128, SHARD], f32)
            nc.sync.dma_start(dinv_rep[:], dinv_loc[:].to_broadcast([128, SHARD]))
            xT_cur = cp.tile([128, SHARD], f32)
            nc.sync.dma_start(xT_cur[:F1, :], xT_l1[:])
            iota_row = cp.tile([128, 128], f32)
            nc.gpsimd.iota(iota_row[:], pattern=[[1, 128]], base=0,
                           channel_multiplier=0,
                           allow_small_or_imprecise_dtypes=True)
            ident = cp.tile([128, 128], f32)
            make_identity(nc, ident[:])
            W1p = cp.tile([F1, HIDDEN], f32)
            nc.sync.dma_start(W1p[:], W1p_d[:])
            W2 = cp.tile([HIDDEN, HIDDEN], f32)
            nc.sync.dma_start(W2[:], W2_d[:])
            W3 = cp.tile([HIDDEN, HIDDEN], f32)
            nc.sync.dma_start(W3[:], W3_d[:])
            Wl = cp.tile([HIDDEN, NUM_CLASSES], f32)
            nc.sync.dma_start(Wl[:], Wl_d[:])
            b1 = cp.tile([HIDDEN, 1], f32)
            nc.sync.dma_start(b1[:], b1_d[:])
            b2 = cp.tile([HIDDEN, 1], f32)
            nc.sync.dma_start(b2[:], b2_d[:])
            b3 = cp.tile([HIDDEN, 1], f32)
            nc.sync.dma_start(b3[:], b3_d[:])
            accT = cp.tile([128, ACC_S], f32)
            nc.vector.memset(accT[:], 0.0)


            u_of = meta["u_of"]

            import os
            # gather chunk in tiles of 128 descriptors; <=896 descriptors per
            # dma_gather (SWDGE descriptor-ring capacity is ~1024)
            CH = int(os.environ.get("GCN_CH", "4"))

            def layer(lnum, tbl, F, W_sb, b_sb, ag_in, ag_out):
                role = (lnum - 1) % 3 + 1
                nA, nB = NBLK * T_A, NBLK * T_B
                aCH = [(s, min(s + CH, nA)) for s in range(0, nA, CH)]
                bCH = [(s, min(s + CH, nB)) for s in range(0, nB, CH)]
                ga, gb = {}, {}
                ai = bi = 0
                for b in range(NBLK):
                    while ai < len(aCH) and aCH[ai][0] < (b + 1) * T_A:
                        s, e = aCH[ai]
                        gt = wp.tile([128, e - s, F], f32, tag="gA", bufs=3,
                                     name=f"gA_{lnum}_{ai}")
                        nc.gpsimd.dma_gather(
                            gt[:], tbl[0:32768, :], idx_sb[:, s * 8:e * 8],
                            (e - s) * BLK, (e - s) * BLK, F)
                        ga[ai] = gt
                        ai += 1
                    while bi < len(bCH) and bCH[bi][0] < (b + 1) * T_B:
                        s, e = bCH[bi]
                        gt = wp.tile([128, e - s, F], f32, tag="gB", bufs=4,
                                     name=f"gB_{lnum}_{bi}")
                        nc.gpsimd.dma_gather(
                            gt[:], tbl[HI_BASE:NPAD, :],
                            idx_sb[:, nA * 8 + s * 8:nA * 8 + e * 8],
                            (e - s) * BLK, (e - s) * BLK, F)
                        gb[bi] = gt
                        bi += 1
                    pz = ps.tile([F, 128], f32, tag="pz", bufs=2,
                                 name=f"pz_{lnum}_{b}")
                    nt = 0
                    for run, gmap, T_r, col0 in (
                        (0, ga, T_A, b * T_A),
                        (1, gb, T_B, b * T_B),
                    ):
                        for t in range(T_r):
                            j = col0 + t                  # stream tile index
                            chunk, sl = j // CH, j % CH
                            dcol = j if run == 0 else nA + j
                            m = wp.tile([128, 128], f32, tag="m", bufs=4,
                                        name=f"m_{lnum}_{b}_{run}_{t}")
                            if M_ON_ACT:
                                u_ab = wp.tile([128, 128], f32, tag="uab",
                                               bufs=4, name=f"u_{lnum}_{b}_{run}_{t}")
                                nc.scalar.activation(
                                    u_ab[:], iota_row[:],
                                    mybir.ActivationFunctionType.Abs,
                                    bias=negdst_sb[:, dcol:dcol + 1])
                                nc.scalar.activation(
                                    m[:], u_ab[:],
                                    mybir.ActivationFunctionType.Relu,
                                    bias=1.0, scale=-1.0)
                            else:
                                nc.vector.tensor_scalar(
                                    out=m[:], in0=iota_row[:],
                                    scalar1=dst_sb[:, dcol:dcol + 1],
                                    scalar2=None, op0=mybir.AluOpType.is_equal)
                            nc.tensor.matmul(
                                pz[:], lhsT=gmap[chunk][:, sl, :], rhs=m[:],
                                start=(nt == 0), stop=(nt == ntile_tot - 1))
                            nt += 1
                    if True:
                        blk_sl = slice(b * BLK, (b + 1) * BLK)
                        s_sb = wp.tile([F, 128], f32, tag="s", bufs=2,
                                       name=f"s_{lnum}_{b}")
                        nc.vector.tensor_tensor(out=s_sb[:], in0=pz[:],
                                                in1=xT_cur[:F, blk_sl],
                                                op=mybir.AluOpType.add)
                        zt = wp.tile([F, 128], f32, tag="zt", bufs=2,
                                     name=f"zt_{lnum}_{b}")
                        nc.vector.tensor_tensor(out=zt[:], in0=s_sb[:],
                                                in1=dinv_rep[:F, blk_sl],
                                                op=mybir.AluOpType.mult)
                        pxn = ps.tile([HIDDEN, 128], f32, tag="pxn", bufs=2,
                                      name=f"pxn_{lnum}_{b}")
                        nc.tensor.matmul(pxn[:], lhsT=W_sb[:], rhs=zt[:],
                                         start=True, stop=True)
                        if role < 3:
                            xnT = wp.tile([HIDDEN, 128], f32, tag="xnT", bufs=2,
                                          name=f"xnT_{lnum}_{b}")
                            nc.scalar.activation(xnT[:], pxn[:],
                                                 mybir.ActivationFunctionType.Relu,
                                                 bias=b_sb[:])
                            nc.vector.tensor_tensor(out=xT_cur[:, blk_sl],
                                                    in0=xnT[:],
                                                    in1=dinv_rep[:, blk_sl],
                                                    op=mybir.AluOpType.mult)
                            ptr = ps.tile([128, HIDDEN], f32, tag="ptr", bufs=2,
                                          name=f"ptr_{lnum}_{b}")
                            nc.tensor.transpose(ptr[:], xT_cur[:, blk_sl], ident[:])
                            tr = wp.tile([128, HIDDEN], f32, tag="tr", bufs=2,
                                         name=f"tr_{lnum}_{b}")
                            nc.scalar.copy(tr[:], ptr[:])
                            nc.sync.dma_start(ag_in[blk_sl, :], tr[:])
                        else:
                            h3T = wp.tile([HIDDEN, 128], f32, tag="xnT", bufs=2,
                                          name=f"h3T_{b}")
                            nc.vector.tensor_scalar(
                                out=h3T[:], in0=pxn[:], scalar1=b_sb[:],
                                scalar2=None, op0=mybir.AluOpType.add)
                            ptr = ps.tile([128, HIDDEN], f32, tag="ptr", bufs=2,
                                          name=f"ptr3_{b}")
                            nc.tensor.transpose(ptr[:], h3T[:], ident[:])
                            tr = wp.tile([128, HIDDEN], f32, tag="tr", bufs=2,
                                         name=f"tr3_{b}")
                            nc.scalar.copy(tr[:], ptr[:])
                            bt = wp.tile([128, BLK], f32, tag="bt", bufs=4,
                                         name=f"bt_{b}")
                            nc.sync.dma_start(bt[:], Bmat_d[:, b * BLK:(b + 1) * BLK])
                            pp = ps.tile([128, HIDDEN], f32, tag="ptr", bufs=2,
                                         name=f"pp_{b}")
                            nc.tensor.matmul(pp[:], lhsT=tr[:], rhs=bt[:],
                                             start=True, stop=True)
                            u = int(u_of[b])
                            nc.vector.tensor_tensor(
                                out=accT[:, u:u + BLK], in0=accT[:, u:u + BLK],
                                in1=pp[:], op=mybir.AluOpType.add)

                if role < 3:
                    nc.gpsimd.collective_compute(
                        "AllGather", mybir.AluOpType.bypass,
                        replica_groups=[list(range(NCORES))],
                        ins=[ag_in[:]], outs=[ag_out[:]])

            ntile_tot = T_A + T_B
            ag_in1 = dp.tile([SHARD, HIDDEN], f32)
            ag_in2 = dp.tile([SHARD, HIDDEN], f32)
            gidx_sb = cp.tile([128, 4], mybir.dt.int32)
            nc.sync.dma_start(gidx_sb[:], gidx_d[:])
            biasm_sb = cp.tile([NUM_CLASSES, ACC_S], f32)
            nc.sync.dma_start(biasm_sb[:], biasmat_d[:])
            pd = dp.tile([PD_ROWS, NUM_CLASSES], f32)
            zt19 = wp.tile([128, PD_ROWS * NUM_CLASSES // 128], f32)
            nc.vector.memset(zt19[:], 0.0)

            for rep in range(repeat):
              ln1, ln2, ln3 = 3 * rep + 1, 3 * rep + 2, 3 * rep + 3
              ag1_out = dp.tile([NPAD, HIDDEN], f32, addr_space="Shared",
                                name=f"ag1_out_{rep}", tag=f"ag1_{rep}")
              ag2_out = dp.tile([NPAD, HIDDEN], f32, addr_space="Shared",
                                name=f"ag2_out_{rep}", tag=f"ag2_{rep}")
              if rep > 0:
                nc.vector.memset(accT[:], 0.0)
              layer(ln1, table1, F1, W1p, b1, ag_in1, ag1_out)
              layer(ln2, ag1_out, HIDDEN, W2, b2, ag_in2, ag2_out)
              layer(ln3, ag2_out, HIDDEN, W3, b3, None, None)

              # ---- pooling tail ----
              nc.sync.dma_start(
                pd[:].rearrange("a b -> (a b)").rearrange("(p f) -> p f", p=128),
                zt19[:])

              for k in range(4):
                py = ps.tile([NUM_CLASSES, 128], f32, tag="ptr", bufs=2,
                             name=f"py_{rep}_{k}")
                nc.tensor.matmul(py[:], lhsT=Wl[:],
                                 rhs=accT[:, k * 128:(k + 1) * 128],
                                 start=True, stop=True)
                y = wp.tile([NUM_CLASSES, 128], f32, tag="s", bufs=2,
                            name=f"y_{rep}_{k}")
                nc.vector.tensor_tensor(
                    out=y[:], in0=py[:],
                    in1=biasm_sb[:, k * 128:(k + 1) * 128],
                    op=mybir.AluOpType.add)
                pyt = ps.tile([128, NUM_CLASSES], f32, tag="pz", bufs=2,
                              name=f"pyt_{rep}_{k}")
                nc.tensor.transpose(pyt[:], y[:], ident[:NUM_CLASSES, :NUM_CLASSES])
                yT = wp.tile([128, NUM_CLASSES], f32, tag="zt", bufs=2,
                             name=f"yT_{rep}_{k}")
                nc.scalar.copy(yT[:], pyt[:])
                nc.gpsimd.indirect_dma_start(
                    out=pd[:],
                    out_offset=bass.IndirectOffsetOnAxis(ap=gidx_sb[:, k:k + 1],
                                                         axis=0),
                    in_=yT[:], in_offset=None)

              pd_red = dp.tile([PD_ROWS, NUM_CLASSES], f32,
                               addr_space="Shared", name=f"pd_red_{rep}",
                               tag=f"pdr_{rep}")
              nc.gpsimd.collective_compute(
                "AllReduce", mybir.AluOpType.add,
                replica_groups=[list(range(NCORES))],
                ins=[pd[:]], outs=[pd_red[:]])
              nc.sync.dma_start(out_d[:], pd_red[0:NUM_GRAPHS, :])

    nc.compile()
    return nc


# --------------------------------------------------------------------------
def kernel(**inputs):
    from concourse import bass_utils

    meta, per_core = _preprocess(**inputs)
    key = (meta["T_A"], meta["T_B"])
    if key not in _cache:
        _cache[key] = _build(meta)
    nc = _cache[key]
    res = bass_utils.run_bass_kernel_spmd(nc, per_core,
                                          core_ids=list(range(NCORES)))
    return np.asarray(res.results[0]["out"], np.float32)
